# revision 84
# baseline (speedup 1.0000x reference)
"""Trainium2 Bass kernel for nn_InverseRecurrentLayer.

Reference computation:
    W_inv = inv(W)
    h[t] = inputs[:, t, :] @ R + bias      # [B, U]  (bias folded into h)
    s_{t+1} = tanh(h[t] + s_t @ Wt),  Wt = W if (t//64)%2==1 else W_inv
    output = states [T, B, U]

Shapes: B=64, T=512, F=512, U=1024. fp32. Data-parallel over batch:
8 cores x B_loc=8.

Per-core plan (v4 — PE-saturated scan, 1.937ms/core in TimelineSim vs
3.23ms for the v1 baseline; PE 94.8% busy):
  Phase A: h = xT.T @ [R; bias] computed as [tb, u] tiles and kept RESIDENT
  in SBUF as bf16 (h_sb, 8MB; no DRAM round-trip). Emitted as a work-item
  stream: 2 h-blocks bootstrap up-front, the remaining matmuls interleave
  one-per-step into the scan's PE bubble so the projection is hidden.

  Phase B: 512-step scan, 4 u-quarters (256 wide) per step:
    - 8 accumulating matmuls per quarter (stationary = sT chunk [128,8],
      moving = W rows [128,256], fp32r => 1 cycle/col; the W stream is the
      hard floor: 8192 cols/step = 3.4us),
    - DVE add: z = ps + h_step (h staged per step to partitions 0..7 by a
      prefetched SBUF-to-SBUF DMA; compute engines need 32-aligned
      partition bases so they cannot read h_sb rows directly),
    - PE transpose of z chunks [8,128] -> ptr [128,8] (identity matmul),
    - ACT tanh(ptr) -> sT quarter tile (f32r, transposed state, no copy).
  Every per-step tensor is a per-quarter tile (whole-tile WAR hazards
  otherwise serialize transposes behind unrelated tanhs); ptr PSUM tiles
  are shared by lifetime-disjoint quarter pairs to fit 8 PSUM banks; the
  emission order software-pipelines each quarter's add/tr/tanh tail under
  later matmul groups so the PE never idles (pstate stays at 2.4 GHz).
  Output is stored transposed [t, 128, 64]; the host driver untransposes.

This environment's walrus encodes at most ONE sync-wait command per
instruction; legalize_waits() hoists extra waits onto InstNoOp carriers,
and the Tile exit barrier is patched to sem-only barriers.
"""
import sys

sys.path.insert(0, "/opt/trn_rl_repo")

import numpy as np
from contextlib import ExitStack

import concourse.bass as bass
import concourse.mybir as mybir
import concourse.tile as tile
from concourse.bass_utils import run_bass_kernel_spmd

# ---------------------------------------------------------------- constants
B, T, F, U = 64, 512, 512, 1024
NCORES = 8
BLOC = B // NCORES          # 8 batch rows per core
KF = F // 128               # 4 k-tiles for the projection
KU = U // 128               # 8 k-tiles (state chunks) for the scan
NQ = 4                      # u-quarters in the scan
QW = U // NQ                # 256 quarter width
F32 = mybir.dt.float32
F32R = mybir.dt.float32r
BF16 = mybir.dt.bfloat16

# ------------------------------------------------- walrus wait legalization


def _patched_drain_and_barrier(self, tick_clock, wait_clock):
    drain_inst = self.nc.sync.drain()
    wait_clock.add_sem_waits(
        drain_inst.ins, tile.ScopedClock({None: tick_clock.global_clock})
    )
    ow = list(drain_inst.ins.sync_info.on_wait or [])
    if len(ow) > 1:
        drain_inst.ins.sync_info.on_wait = ow[:1]
        for w in ow[1:]:
            d2 = self.nc.sync.drain()
            d2.ins.sync_info = mybir.SyncInfo(on_wait=[w], on_update=[])
    self.nc.all_engine_barrier(sem_only=True)
    popped = self.nc._tile_sem_poison_stack.pop()
    assert popped is self._sem_poison
    self.nc.clear_and_free_semaphores(list(self.sems.allocated().values()))
    self.nc.all_engine_barrier(sem_only=True)


tile.TileContext._drain_and_barrier = _patched_drain_and_barrier


def legalize_waits(nc):
    """Split multi-wait instructions: keep 1 wait, hoist the rest onto
    InstNoOp carriers inserted just before, on the same engine."""
    n = 0
    for fn in nc.m.functions:
        for blk in fn.blocks:
            out = []
            for inst in blk.instructions:
                si = inst.sync_info
                if si is not None and si.on_wait and len(si.on_wait) > 1:
                    waits = list(si.on_wait)
                    for w in waits[:-1]:
                        n += 1
                        nop = mybir.InstNoOp(
                            name=f"waitcar-{n}-{inst.name}",
                            engine=inst.engine,
                            ins=[],
                            outs=[],
                            sync_info=mybir.SyncInfo(on_wait=[w], on_update=[]),
                        )
                        nc.register_instruction(nop)
                        out.append(nop)
                    si.on_wait = waits[-1:]
                out.append(inst)
            blk.instructions[:] = out
    return n


# ------------------------------------------------------------ device kernel

LABELS = {}


def _lbl(bi, label):
    try:
        LABELS[bi.ins.name] = label
    except Exception:
        pass
    return bi


def build_kernel(t_steps=T, with_bias=True):
    nc = bass.Bass("TRN2", target_bir_lowering=False, debug=False)
    tb = BLOC * t_steps
    m_tiles = tb // 128          # 128-col tb tiles in phase A
    h_blocks = m_tiles           # h_sb u-blocks, one per 16 steps

    xT_d = nc.dram_tensor("xT", [F, tb], F32R, kind="ExternalInput").ap()
    Ra_d = nc.dram_tensor("Ra", [F + 1, U], F32R, kind="ExternalInput").ap()
    W2_d = nc.dram_tensor("W2", [2, U, U], F32R, kind="ExternalInput").ap()
    x0T_d = nc.dram_tensor("x0T", [128, KU], F32, kind="ExternalInput").ap()
    id8_d = nc.dram_tensor("id8", [8, 8], F32, kind="ExternalInput").ap()
    out_d = nc.dram_tensor("out", [t_steps, 128, KU * BLOC], F32,
                           kind="ExternalOutput").ap()

    with tile.TileContext(nc) as tc, ExitStack() as ctx:
        const = ctx.enter_context(tc.tile_pool(name="const", bufs=1))
        # resident tensors
        w_sb = const.tile([128, 2 * KU * 1024], F32R)      # 8 MB, both phases
        ra_sb = const.tile([128, KF * 1024], F32R)         # R k-tiles
        rb_sb = const.tile([1, 1024], F32R)                # bias row of Ra
        h_sb = const.tile([128, h_blocks * 1024], BF16)    # resident h, 8 MB
        xo_sb = const.tile([1, 128], F32R)                 # ones row (const)
        x0_sb = const.tile([128, KU], F32)
        id8 = const.tile([8, 8], F32)

        # phase-A inputs first so the W bulk doesn't hog the DMA queues
        # during bootstrap
        for k in range(KF):
            nc.sync.dma_start(
                ra_sb[:, k * 1024:(k + 1) * 1024], Ra_d[k * 128:(k + 1) * 128, :]
            )
        nc.sync.dma_start(rb_sb[:, :], Ra_d[F:F + 1, :])
        nc.sync.dma_start(x0_sb[:, :], x0T_d[:, :])
        nc.sync.dma_start(id8[:, :], id8_d[:, :])
        for p in range(2):
            for k in range(KU):
                nc.sync.dma_start(
                    w_sb[:, (p * KU + k) * 1024:(p * KU + k + 1) * 1024],
                    W2_d[p, k * 128:(k + 1) * 128, :],
                )
        # constant ones row for the bias matmul
        ones_f = xo_sb.bitcast(F32)
        nc.vector.memset(ones_f, 1.0)

        # ---------------- phase A: h = xT.T @ [R; bias] -> bf16 SBUF.
        # Structured as a work-item generator: the first blocks are emitted
        # up-front, the rest interleave into the scan's per-step PE bubble
        # (one matmul per step) so the projection costs no extra wall time.
        MQ = min(4, m_tiles)      # m-tiles fetched per DMA batch
        xpool = ctx.enter_context(tc.tile_pool(name="xstage", bufs=2))
        papool = ctx.enter_context(
            tc.tile_pool(name="psum_proj", bufs=1, space="PSUM")
        )
        pa_state = {}

        def phase_a_items():
            for mq in range(m_tiles // MQ):
                def dma_item(mq=mq):
                    xa = xpool.tile([128, KF, MQ * 128], F32R, tag="xa",
                                    name="xa")
                    for k in range(KF):
                        # off the SP queue (it carries the out/h DMAs)
                        nc.scalar.dma_start(
                            xa[:, k, :],
                            xT_d[k * 128:(k + 1) * 128,
                                 mq * MQ * 128:(mq + 1) * MQ * 128],
                        )
                    pa_state["xa"] = xa
                yield ("dma", dma_item)
                for j in range(MQ):
                    m = mq * MQ + j
                    for n in range(2):
                        n_mms = KF + 1 if with_bias else KF
                        for k in range(n_mms):
                            def mm_item(j=j, n=n, k=k, last=(k == n_mms - 1)):
                                if k == 0:
                                    pa_state["ps"] = papool.tile(
                                        [128, 512], F32, tag="psA", name="psA"
                                    )
                                if k < KF:
                                    nc.tensor.matmul(
                                        pa_state["ps"][:],
                                        pa_state["xa"][:, k,
                                                       j * 128:(j + 1) * 128],
                                        ra_sb[:, k * 1024 + n * 512:
                                              k * 1024 + n * 512 + 512],
                                        start=(k == 0),
                                        stop=last,
                                    )
                                else:
                                    nc.tensor.matmul(
                                        pa_state["ps"][:],
                                        xo_sb[:, :],
                                        rb_sb[:, n * 512:n * 512 + 512],
                                        start=False,
                                        stop=True,
                                    )
                            yield ("mm", mm_item)

                        def copy_item(m=m, n=n):
                            # Pool/GPSIMD cannot read PSUM on HW; DVE has an
                            # idle window after the step's four adds, ACT
                            # copies would delay the carried tanh q2
                            dst = h_sb[:, m * 1024 + n * 512:
                                       m * 1024 + n * 512 + 512]
                            nc.vector.tensor_copy(dst, pa_state["ps"][:])
                        yield ("copy", copy_item)

        pa_iter = phase_a_items()
        pa_next = [next(pa_iter)]

        def pa_pull(kinds, limit=1):
            done = 0
            while done < limit and pa_next[0] is not None:
                kind, fn = pa_next[0]
                if kind not in kinds:
                    return
                fn()
                done += 1
                pa_next[0] = next(pa_iter, None)

        # bootstrap: h blocks 0..1 (1 dma batch + 2 blocks of items)
        blk_items = 2 * ((KF + 1 if with_bias else KF) + 1)
        pa_pull(("dma", "mm", "copy"), limit=1 + 2 * blk_items)

        # ---------------- phase B: the scan
        # Every per-step tensor is split per u-quarter into its own tile:
        # the tile framework treats write-after-read hazards at whole-tile
        # granularity, and shared tiles serialize transposes behind
        # unrelated tanhs. PSUM budget: 4 mm tiles + 4 ptr tiles = 8 banks.
        mmp = ctx.enter_context(tc.tile_pool(name="psum_mm", bufs=1, space="PSUM"))
        trp = ctx.enter_context(tc.tile_pool(name="psum_tr", bufs=1, space="PSUM"))
        zpool = ctx.enter_context(tc.tile_pool(name="z", bufs=3))
        spool = ctx.enter_context(tc.tile_pool(name="sT", bufs=3))
        hpool = ctx.enter_context(tc.tile_pool(name="hstep", bufs=3))
        QC = 2 * BLOC                                    # 16 cols per quarter

        # initial transposed state from x0 (per quarter)
        sT_prev = []
        for q in range(NQ):
            sq = spool.tile([128, QC], F32R, tag=f"sT{q}", name=f"sTi{q}")
            for c in range(2):
                nc.vector.tensor_copy(
                    sq[:, c * BLOC:(c + 1) * BLOC],
                    x0_sb[:, 2 * q + c:2 * q + c + 1].broadcast_to([128, BLOC]),
                )
            sT_prev.append(sq)

        ps_t = [None] * NQ          # live psum tiles per quarter
        z_t = [None] * NQ           # live z tiles per quarter
        ptr_t = [None] * NQ         # live transposed-preact tiles per quarter
        sT_of = {-1: sT_prev}       # step -> [4 quarter state tiles]

        def emit_mms(t, q, ks, sT_src):
            p = 1 if (t // 64) % 2 == 1 else 0
            for k in ks:
                if k == 0:
                    ps_t[q] = mmp.tile([BLOC, QW], F32, tag=f"mm{q}",
                                       name=f"ps{q}",
                                       bufs=2 if q == 3 else 1)
                wc = (p * KU + k) * 1024 + q * QW
                sq = sT_src[k // 2]
                _lbl(nc.tensor.matmul(
                    ps_t[q][:],
                    sq[:, (k % 2) * BLOC:(k % 2 + 1) * BLOC],
                    w_sb[:, wc:wc + QW],
                    start=(k == 0),
                    stop=(k == KU - 1),
                ), f"mm t{t} q{q} k{k}")

        h_step = {}                 # step -> staged [8, 1024] h tile

        def emit_hprefetch(t):
            # compute engines need 32-aligned partition bases, so each step's
            # h rows are DMA-staged (partition-free) to partitions 0..7 one
            # step ahead of use
            if t >= t_steps:
                return
            ht = hpool.tile([BLOC, U], BF16, tag="hstep", name=f"hs{t % 3}")
            prow = (t % 16) * 8
            blk = t // 16
            nc.sync.dma_start(
                ht[:], h_sb[prow:prow + 8, blk * 1024:(blk + 1) * 1024]
            )
            h_step[t] = ht
            h_step.pop(t - 3, None)

        def emit_add(t, q, engine, split=False):
            z_t[q] = zpool.tile([BLOC, QW], F32, tag=f"z{q}", name=f"z{q}")
            if split:
                # two half-adds so the first transpose can start ~140ns
                # earlier (chunk 2q's chain is the cycle-critical path)
                for h in range(2):
                    _lbl(engine.tensor_add(
                        z_t[q][:, h * 128:(h + 1) * 128],
                        ps_t[q][:, h * 128:(h + 1) * 128],
                        h_step[t][:, q * QW + h * 128:q * QW + (h + 1) * 128],
                    ), f"add t{t} q{q}{'ab'[h]}")
            else:
                hsl = h_step[t][:, q * QW:(q + 1) * QW]
                _lbl(engine.tensor_add(z_t[q][:], ps_t[q][:], hsl),
                     f"add t{t} q{q}")

        # ptr PSUM tiles are shared by quarter pairs with disjoint lifetime
        # windows — q0+q3 ("Y") and q1+q2 ("X") — to fit 8 PSUM banks while
        # double-buffering the q2/q3 matmul tiles.
        ptr_pair = {"X": {}, "Y": {}}
        PAIR = {0: ("Y", 0), 3: ("Y", QC), 1: ("X", 0), 2: ("X", QC)}

        def emit_chain(t, q, split=False, trs_only=False, tanh_only=False):
            # transposes + tanh + store for quarter q of step t
            pk, po = PAIR[q]
            if po == 0:
                ptr_pair[pk][t] = trp.tile([128, 2 * QC], F32, tag=f"ptr{pk}",
                                           name=f"ptr{pk}")
                ptr_pair[pk].pop(t - 2, None)
            ptr = ptr_pair[pk][t]
            if not tanh_only:
                for half in range(2):
                    _lbl(nc.tensor.transpose(
                        ptr[:, po + half * BLOC:po + (half + 1) * BLOC],
                        z_t[q][:, half * 128:(half + 1) * 128],
                        id8[:, :],
                    ), f"tr t{t} c{2 * q + half}")
            if trs_only:
                return
            sq = spool.tile([128, QC], F32R, tag=f"sT{q}", name=f"sT{q}")
            sT_of[t][q] = sq
            for half in range(2):
                if split:
                    _lbl(nc.scalar.activation(
                        sq[:, half * BLOC:(half + 1) * BLOC],
                        ptr[:, po + half * BLOC:po + (half + 1) * BLOC],
                        mybir.ActivationFunctionType.Tanh,
                    ), f"tanh t{t} q{q}{'ab'[half]}")
            if not split:
                _lbl(nc.scalar.activation(
                    sq[:, :],
                    ptr[:, po:po + QC],
                    mybir.ActivationFunctionType.Tanh,
                ), f"tanh t{t} q{q}")
            nc.sync.dma_start(out_d[t, :, q * QC:(q + 1) * QC],
                              sq[:].bitcast(F32))

        emit_hprefetch(0)
        emit_hprefetch(1)
        for t in range(t_steps):
            sT_of[t] = [None] * NQ
            src = sT_of[t - 1]
            emit_hprefetch(t + 2)
            # src[c//2]: quarter tiles of step t-1's state; quarters 2,3 are
            # finished below (emit_chain(t-1, 2/3)) before any k4..7 matmul
            # consumes them.
            for q in range(NQ):
                emit_mms(t, q, (0, 1), src)              # all k0,k1
            if t > 0:
                emit_chain(t - 1, 2)                     # trs c4,c5 + tanh
                emit_chain(t - 1, 3)                     # trs c6,c7 + tanh
            for q in range(3):
                emit_mms(t, q, (2, 3), src)              # q0..q2 k2,k3
            emit_mms(t, 0, (4, 5, 6, 7), src)            # q0[k4..7]
            emit_add(t, 0, nc.vector)                    # add q0 (DVE)
            emit_mms(t, 3, (2, 3), src)                  # q3[k2,3] deferred
            emit_mms(t, 1, (4, 5, 6, 7), src)            # q1[k4..7]
            emit_add(t, 1, nc.vector)                    # add q1 (DVE)
            emit_mms(t, 2, (4, 5), src)                  # q2[k4,5]
            pa_pull(("mm",), limit=1)                    # phase-A filler mm
            emit_chain(t, 0)                             # trs c0,c1 + tanh q0
            emit_mms(t, 2, (6, 7), src)                  # q2[k6,7]
            emit_add(t, 2, nc.vector)                    # add q2 (DVE)
            emit_mms(t, 3, (4, 5, 6), src)               # q3[k4..6]
            emit_chain(t, 1)                             # trs c2,c3 + tanh q1
            emit_mms(t, 3, (7,), src)                    # q3[k7]
            emit_add(t, 3, nc.vector)                    # add q3 (DVE)
            pa_pull(("copy", "dma"), limit=2)            # phase-A non-PE items
            sT_of.pop(t - 2, None)

        # epilogue: finish last step's quarters 2,3
        tl = t_steps - 1
        emit_chain(tl, 2)
        emit_chain(tl, 3)

    legalize_waits(nc)
    return nc


# -------------------------------------------------------------- host driver
_CACHE = {}


def _get_nc(t_steps, with_bias=True):
    key = (t_steps, with_bias)
    if key not in _CACHE:
        _CACHE[key] = build_kernel(t_steps, with_bias)
    return _CACHE[key]


def kernel(inputs, R, W, bias, x0, t_steps=None, n_cores=NCORES, trace=False,
           trace_kw=None):
    t_steps = t_steps or inputs.shape[1]
    inputs = np.ascontiguousarray(inputs, dtype=np.float32)
    R = np.asarray(R, dtype=np.float32)
    W = np.asarray(W, dtype=np.float32)
    bias = np.asarray(bias, dtype=np.float32)
    x0 = np.asarray(x0, dtype=np.float32)

    W_inv = np.linalg.inv(W)
    W2 = np.stack([W_inv, W]).astype(np.float32)        # phase 0 = W_inv
    Ra = np.concatenate([R, bias[None, :]], axis=0)      # [F+1, U]
    x0T = np.ascontiguousarray(x0.reshape(KU, 128).T)    # [128, KU]
    id8 = np.eye(8, dtype=np.float32)

    in_maps = []
    for c in range(n_cores):
        xc = inputs[c * BLOC:(c + 1) * BLOC, :t_steps, :]   # [BLOC, t, F]
        # xT[f, t*BLOC+b] (t-major cols)
        xT = np.ascontiguousarray(
            xc.transpose(2, 1, 0).reshape(F, BLOC * t_steps)
        )
        in_maps.append(
            {"xT": xT, "Ra": Ra, "W2": W2, "x0T": x0T, "id8": id8}
        )

    nc = _get_nc(t_steps, with_bias=bool(np.any(bias)))
    try:
        res = run_bass_kernel_spmd(
            nc, in_maps, core_ids=list(range(n_cores)), trace=trace,
            **(trace_kw or {}),
        )
    except Exception:
        # transient device wedges (NRT_EXEC_UNIT_UNRECOVERABLE) usually
        # clear on a retry
        res = run_bass_kernel_spmd(
            nc, in_maps, core_ids=list(range(n_cores)), trace=trace,
            **(trace_kw or {}),
        )
    kernel.last_result = res
    kernel.last_nc = nc
    # assemble [T, B, U]: per-core out is [t, 128, KU*BLOC] transposed state
    full = np.empty((t_steps, n_cores * BLOC, U), np.float32)
    for c in range(n_cores):
        arr = res.results[c]["out"]                      # [t, 128, 64]
        full[:, c * BLOC:(c + 1) * BLOC, :] = (
            arr.reshape(t_steps, 128, KU, BLOC)
            .transpose(0, 3, 2, 1)
            .reshape(t_steps, BLOC, U)
        )
    return full
